# revision 1
# baseline (speedup 1.0000x reference)
"""Trainium2 8-core fused attention kernel (QKV proj + RMSNorm + RoPE + SDPA + out proj).

Sharding: tensor-parallel over heads. Each of the 8 cores computes 2 of the 16
heads end-to-end (QKV projection with its Wqkv column shard, per-head RMSNorm +
RoPE, full softmax attention), then an AllToAll redistributes the per-head
attention outputs so every core holds all 1024 attention channels for 1/8 of
the tokens and applies the full Wout to its token shard.

Self-contained: hardcodes all shapes from the problem spec.
"""
import os
import sys
import types

import numpy as np
import ml_dtypes

sys.path.insert(0, "/opt/trn_rl_repo")

from concourse import bass, bacc, tile, mybir  # noqa: E402
from concourse.bass_utils import run_bass_kernel_spmd  # noqa: E402
from concourse.masks import make_identity  # noqa: E402

B, N, C, H, D = 2, 4096, 1024, 16, 64
NCORES = 8
TOK = B * N            # 8192 global tokens
NB = N // 128          # 32 token tiles per batch
NMACRO = N // 256      # 16 macro tiles (256 tok) per batch
QTILE = 512
NQT = N // QTILE       # 8 q tiles per batch
KC = N // 128          # 32 key chunks per batch
SHARD = TOK // NCORES  # 1024 tokens per core shard
EPS = 1e-6

F32 = mybir.dt.float32
F32R = mybir.dt.float32r
BF16 = mybir.dt.bfloat16
ALU = mybir.AluOpType
ACTF = mybir.ActivationFunctionType

_CACHE = {}
_LAST_RESULT = None


def _install_profile_shim():
    """trn_boot skips the NTFF hook when antenv.axon_hooks is missing; supply it."""
    try:
        import antenv
        if getattr(antenv, "axon_hooks", None) is not None:
            return
        from trn_agent_boot.trn_boot import _ntff_profile_via_ctypes
        hook = _ntff_profile_via_ctypes("/opt/axon/libaxon_pjrt.so")
        if hook is None:
            return
        mod = types.ModuleType("antenv.axon_hooks")
        state = {"hook": hook}
        mod.get_axon_ntff_profile_hook = lambda: state["hook"]
        mod.set_axon_ntff_profile_hook = lambda h: state.__setitem__("hook", h)
        sys.modules["antenv.axon_hooks"] = mod
        antenv.axon_hooks = mod
    except Exception:
        pass


def _build_graph():
    nc = bacc.Bacc("TRN2", target_bir_lowering=False, debug=False,
                   enable_asserts=True, num_devices=NCORES)

    hsT_d = nc.dram_tensor("hsT", [C, TOK], BF16, kind="ExternalInput")
    wqkv_d = nc.dram_tensor("wqkv", [C, 384], BF16, kind="ExternalInput")
    trigc_d = nc.dram_tensor("trigc", [N, 256], BF16, kind="ExternalInput")
    trigs_d = nc.dram_tensor("trigs", [N, 256], BF16, kind="ExternalInput")
    wout_d = nc.dram_tensor("wout", [C, C], BF16, kind="ExternalInput")
    out_d = nc.dram_tensor("out", [SHARD, C], F32, kind="ExternalOutput")

    with tile.TileContext(nc) as tc:
        with tc.tile_pool(name="const", bufs=1) as constp, \
             tc.tile_pool(name="dram", bufs=1, space="DRAM") as dram:
            # resident weights
            wqkv_sb = constp.tile([128, 8, 384], BF16)
            nc.sync.dma_start(wqkv_sb[:], wqkv_d.ap().rearrange("(a p) n -> p a n", p=128))
            ident = constp.tile([128, 128], BF16)
            make_identity(nc, ident[:])
            ones_f = constp.tile([65, 64], F32)
            nc.vector.memset(ones_f[:], 1.0)
            ones_sb = constp.tile([65, 64], F32R)
            nc.vector.tensor_copy(ones_sb[:], ones_f[:])

            a2a_in = [dram.tile([NCORES, 128, SHARD // 2], BF16,
                                  name=f"a2a_in{h}", tag=f"a2a_in{h}") for h in range(2)]
            a2a_out = [dram.tile([NCORES, 128, SHARD // 2], BF16,
                                   name=f"a2a_out{h}", tag=f"a2a_out{h}") for h in range(2)]

            with tc.tile_pool(name="batch", bufs=1) as bp, \
                 tc.tile_pool(name="stream", bufs=6) as sp, \
                 tc.tile_pool(name="work", bufs=3) as wp, \
                 tc.tile_pool(name="probsp", bufs=4) as pp, \
                 tc.tile_pool(name="ps1", bufs=2, space="PSUM") as ps1, \
                 tc.tile_pool(name="pssc", bufs=2, space="PSUM") as pssc, \
                 tc.tile_pool(name="psat", bufs=1, space="PSUM") as psat:

                qT = [bp.tile([128, N], BF16, name=f"qT{b}", tag=f"qT{b}") for b in range(B)]
                kT = [bp.tile([128, N], BF16, name=f"kT{b}", tag=f"kT{b}") for b in range(B)]
                vsb = [bp.tile([128, NB, 2, 65], BF16, name=f"v{b}", tag=f"v{b}")
                       for b in range(B)]
                atn = [[bp.tile([64, N], BF16, name=f"at{b}{h}", tag=f"at{b}{h}")
                        for h in range(2)] for b in range(B)]
                for b in range(B):
                    nc.vector.memset(vsb[b][:, :, :, 64:65], 1.0)

                # ---------------- Stage A: QKV + RMSNorm + RoPE + transposes ----
                def emit_A(b, mt):
                    hs_t = []
                    for cc in range(8):
                        t = sp.tile([128, 256], BF16, name=f"hs{cc}", tag=f"hs{cc}")
                        nc.sync.dma_start(
                            t[:], hsT_d.ap()[cc * 128:(cc + 1) * 128,
                                             b * N + mt * 256: b * N + (mt + 1) * 256])
                        hs_t.append(t)
                    trigC = sp.tile([128, 2, 256], BF16, name="trigC", tag="trigC")
                    trigS = sp.tile([128, 2, 256], BF16, name="trigS", tag="trigS")
                    for dst, dt_ in ((trigC, trigc_d), (trigS, trigs_d)):
                        nc.sync.dma_start(
                            dst[:], dt_.ap()[mt * 256:(mt + 1) * 256, :]
                            .rearrange("(s p) d -> p s d", p=128))

                    for sub in range(2):
                        tt = mt * 2 + sub  # token tile index within batch
                        ps_qkv = ps1.tile([128, 384], F32, name="ps_qkv", tag="ps1")
                        for cc in range(8):
                            nc.tensor.matmul(
                                ps_qkv[:],
                                lhsT=hs_t[cc][:, sub * 128:(sub + 1) * 128],
                                rhs=wqkv_sb[:, cc, :],
                                start=(cc == 0), stop=(cc == 7))

                        # q/k block to SBUF; in the batch-0 prefix ACT is idle, so
                        # route the copies there (Copy needs no ACT table switch)
                        cpeng = nc.scalar if (b == 0 and mt < NMACRO // 2) else nc.vector
                        qk_sb = wp.tile([128, 256], F32, name="qk_sb", tag="qk_sb", bufs=3)
                        if cpeng is nc.scalar:
                            nc.scalar.copy(qk_sb[:], ps_qkv[:, 0:256])
                        else:
                            nc.vector.tensor_copy(qk_sb[:], ps_qkv[:, 0:256])
                        nc.vector.tensor_copy(
                            vsb[b][:, tt, :, 0:64],
                            ps_qkv[:, 256:384].rearrange("p (h d) -> p h d", h=2))
                        # sumsq for (q h0, q h1, k h0, k h1) -> [128, 4]
                        sq = wp.tile([128, 256], F32, name="sq", tag="sq", bufs=3)
                        ssq4 = wp.tile([128, 4], F32, name="ssq4", tag="ssq4")
                        nc.vector.tensor_mul(sq[:], qk_sb[:], qk_sb[:])
                        nc.vector.tensor_reduce(
                            ssq4[:], sq[:].rearrange("p (a e) -> p a e", a=4),
                            axis=mybir.AxisListType.X, op=ALU.add)
                        # rinv = 8/sqrt(ssq): bit-trick seed + 1 Newton step
                        # (the /64 mean and *8 fold together; eps negligible here)
                        yv = wp.tile([128, 4], F32, name="yv", tag="yv")
                        with nc.allow_low_precision(reason="rsqrt newton seed"):
                            nc.vector.tensor_scalar(
                                out=yv[:].bitcast(mybir.dt.int32),
                                in0=ssq4[:].bitcast(mybir.dt.int32),
                                scalar1=1, scalar2=None, op0=ALU.arith_shift_right)
                            nc.vector.tensor_scalar(
                                out=yv[:].bitcast(mybir.dt.int32),
                                in0=yv[:].bitcast(mybir.dt.int32),
                                scalar1=-1, scalar2=0x5F3759DF,
                                op0=ALU.mult, op1=ALU.add)
                        tn = wp.tile([128, 4], F32, name="tn", tag="tn")
                        nc.vector.tensor_mul(tn[:], yv[:], yv[:])
                        nc.vector.tensor_mul(tn[:], tn[:], ssq4[:])
                        nc.vector.tensor_scalar(out=tn[:], in0=tn[:],
                                                scalar1=-4.0, scalar2=12.0,
                                                op0=ALU.mult, op1=ALU.add)
                        nc.vector.tensor_mul(yv[:], yv[:], tn[:])
                        # normalize all 4 groups at once (free-dim broadcast of rinv)
                        qn2 = wp.tile([128, 256], F32, name="qn2", tag="qn2", bufs=3)
                        nc.vector.tensor_tensor(
                            out=qn2[:].rearrange("p (a e) -> p a e", a=4),
                            in0=qk_sb[:].rearrange("p (a e) -> p a e", a=4),
                            in1=yv[:].unsqueeze(2).broadcast_to([128, 4, 64]),
                            op=ALU.mult)
                        d_qk = wp.tile([128, 256], F32, name="d_qk", tag="d_qk", bufs=3)
                        nc.vector.tensor_mul(d_qk[:], qn2[:], trigC[:, sub, :])
                        trot = wp.tile([128, 256], F32, name="trot", tag="trot", bufs=3)
                        v4 = qn2[:].rearrange("p (a e) -> p a e", a=8)
                        s4 = trigS[:, sub, :].rearrange("p (a e) -> p a e", a=8)
                        t4 = trot[:].rearrange("p (a e) -> p a e", a=8)
                        nc.vector.tensor_mul(t4[:, 0:8:2, :], v4[:, 1:8:2, :],
                                             s4[:, 0:8:2, :])
                        nc.vector.tensor_mul(t4[:, 1:8:2, :], v4[:, 0:8:2, :],
                                             s4[:, 1:8:2, :])
                        d_bf = wp.tile([128, 256], BF16, name="d_bf", tag="d_bf", bufs=3)
                        nc.vector.tensor_add(d_bf[:], d_qk[:], trot[:])
                        for half, dstname in ((0, "q"), (1, "k")):
                            ps_t = ps1.tile([128, 128], BF16, name="ps_t", tag="ps1")
                            nc.tensor.transpose(
                                ps_t[:], d_bf[:, half * 128:(half + 1) * 128], ident[:])
                            dst = qT[b] if dstname == "q" else kT[b]
                            if cpeng is nc.scalar:
                                nc.scalar.copy(dst[:, tt * 128:(tt + 1) * 128], ps_t[:])
                            else:
                                nc.vector.tensor_copy(dst[:, tt * 128:(tt + 1) * 128], ps_t[:])

                # ---------------- Stage B: attention --------------------------
                # at_acc: persistent accumulators for split-half q-tiles (batch 0
                # qt 0-3) so attention can start when stage A is half done.
                at_acc = [bp.tile([65, 2 * QTILE], F32R, name=f"at_acc{q}",
                                  tag=f"at_acc{q}") for q in range(NQT)]

                def emit_B(b, qt, kc_lo=0, kc_hi=KC - 1, acc=None):
                    at_ps = psat.tile([65, 2 * QTILE], F32, name="at_ps", tag="psat",
                                      bufs=1)
                    at_ps_h = [at_ps[:, h * QTILE:(h + 1) * QTILE] for h in range(2)]
                    prev_pr = None
                    for kc in range(kc_lo, kc_hi + 1):
                        ps_s = pssc.tile([128, 2 * QTILE], F32, name="ps_s", tag="pssc")
                        for hh in range(2):
                            nc.tensor.matmul(
                                ps_s[:, hh * QTILE:(hh + 1) * QTILE],
                                lhsT=kT[b][64 * hh:64 * (hh + 1),
                                           kc * 128:(kc + 1) * 128],
                                rhs=qT[b][64 * hh:64 * (hh + 1),
                                          qt * QTILE:(qt + 1) * QTILE],
                                start=True, stop=True)
                        pr = pp.tile([128, 2 * QTILE], BF16, name="pr", tag="pr",
                                     bufs=6)
                        nc.scalar.activation(pr[:], ps_s[:], ACTF.Exp,
                                             bias=0.0, scale=0.125)
                        if prev_pr is not None:
                            pkc, ppr = prev_pr
                            for hh in range(2):
                                nc.tensor.matmul(
                                    at_ps_h[hh],
                                    lhsT=vsb[b][:, pkc, hh, :],
                                    rhs=ppr[:, hh * QTILE:(hh + 1) * QTILE],
                                    start=(pkc == kc_lo), stop=(pkc == kc_hi))
                        prev_pr = (kc, pr)
                    pkc, ppr = prev_pr
                    for hh in range(2):
                        nc.tensor.matmul(
                            at_ps_h[hh],
                            lhsT=vsb[b][:, pkc, hh, :],
                            rhs=ppr[:, hh * QTILE:(hh + 1) * QTILE],
                            start=(pkc == kc_lo), stop=(pkc == kc_hi))
                    # evacuate attn psum to sbuf (frees psat for the next q-tile);
                    # for split q-tiles the evacuation doubles as accumulate.
                    if kc_hi < KC - 1:
                        # partial segment: accumulate and come back later
                        for hh in range(2):
                            dst = acc[:, hh * QTILE:(hh + 1) * QTILE]
                            if kc_lo == 0:
                                nc.vector.tensor_copy(dst, at_ps_h[hh])
                            else:
                                nc.vector.tensor_add(dst, dst, at_ps_h[hh])
                        return
                    if acc is not None and kc_lo > 0:
                        at_sb = acc
                        for hh in range(2):
                            dst = at_sb[:, hh * QTILE:(hh + 1) * QTILE]
                            nc.vector.tensor_add(dst, dst, at_ps_h[hh])
                    else:
                        at_sb = wp.tile([65, 2 * QTILE], F32R, name="at_sb", tag="at_sb",
                                        bufs=2)
                        for hh in range(2):
                            nc.vector.tensor_copy(at_sb[:, hh * QTILE:(hh + 1) * QTILE],
                                                  at_ps_h[hh])
                    for hh in range(2):
                        aps = at_sb[:, hh * QTILE:(hh + 1) * QTILE]
                        ps_bc = pssc.tile([64, QTILE], F32, name="ps_bc", tag="pssc")
                        nc.tensor.matmul(
                            ps_bc[:],
                            lhsT=ones_sb[64:65, :],
                            rhs=aps[64:65, :],
                            start=True, stop=True)
                        rbc = wp.tile([64, QTILE], F32, name="rbc", tag="rbc", bufs=2)
                        nc.vector.tensor_copy(rbc[:], ps_bc[:])
                        rbcr = wp.tile([64, QTILE], F32, name="rbcr", tag="rbcr", bufs=2)
                        nc.vector.reciprocal_approx_fast(out=rbcr[:], in_=rbc[:])
                        nc.gpsimd.tensor_mul(
                            atn[b][hh][:, qt * QTILE:(qt + 1) * QTILE],
                            aps[0:64, :], rbcr[:])

                # Emission order: first half of A(b0); first-half attention for
                # qt 0-3 of b0; rest of A(b0); remaining b0 attention interleaved
                # with A(b1); then B(b1).
                for mt in range(NMACRO // 2):
                    emit_A(0, mt)
                for qt in range(NQT):
                    emit_A(0, NMACRO // 2 + qt)
                    emit_B(0, qt, 0, KC // 2 - 1, acc=at_acc[qt])
                for qt in range(NQT):
                    emit_A(1, 2 * qt)
                    emit_A(1, 2 * qt + 1)
                    emit_B(0, qt, KC // 2, KC - 1, acc=at_acc[qt])
                for qt in range(NQT):
                    emit_B(1, qt)


                # a2a input staging: one collective per batch, 512 tokens per
                # destination core, so batch-0 exchange + projection overlap the
                # whole batch-1 attention phase.
                for ha in range(2):
                    for d in range(NCORES):
                        base = d * 512
                        nc.sync.dma_start(a2a_in[ha][d, 0:64, :],
                                          atn[ha][0][:, base:base + 512])
                        nc.sync.dma_start(a2a_in[ha][d, 64:128, :],
                                          atn[ha][1][:, base:base + 512])

            for ha in range(2):
                nc.gpsimd.collective_compute(
                    "AllToAll", ALU.bypass,
                    ins=[a2a_in[ha][:].opt()], outs=[a2a_out[ha][:].opt()],
                    replica_groups=[list(range(NCORES))])

            # ---------------- Stage C: output projection ----------------------
            with tc.tile_pool(name="cstage", bufs=1) as cp, \
                 tc.tile_pool(name="cwork", bufs=2) as cw, \
                 tc.tile_pool(name="psC", bufs=2, space="PSUM") as psC:
                wout_sb = cp.tile([128, 8, C], BF16)
                nc.sync.dma_start(wout_sb[:], wout_d.ap().rearrange("(a p) n -> p a n", p=128))
                atf = cp.tile([128, 8, SHARD], BF16)
                for ha in range(2):
                    nc.sync.dma_start(atf[:, :, ha * 512:(ha + 1) * 512],
                                      a2a_out[ha][:].transpose([1, 0, 2]))
                for ttk in range(SHARD // 128):
                    ostage = cw.tile([128, C], F32, name="ostage", tag="ostage")
                    for half in range(2):
                        ps_o = psC.tile([128, 512], F32, name="ps_o", tag="psC")
                        for cc in range(8):
                            nc.tensor.matmul(
                                ps_o[:],
                                lhsT=atf[:, cc, ttk * 128:(ttk + 1) * 128],
                                rhs=wout_sb[:, cc, half * 512:(half + 1) * 512],
                                start=(cc == 0), stop=(cc == 7))
                        nc.vector.tensor_copy(ostage[:, half * 512:(half + 1) * 512], ps_o[:])
                    nc.sync.dma_start(out_d.ap()[ttk * 128:(ttk + 1) * 128, :], ostage[:])

    nc.compile()
    return nc


def _fold_sin(sin, g):
    out = np.empty_like(sin)
    out[:, :32] = -sin[:, :32] * g[32:]
    out[:, 32:] = sin[:, 32:] * g[:32]
    return out


def kernel(hidden_states, cos, sin, Wqkv, Wout, gq, gk):
    global _LAST_RESULT
    _install_profile_shim()

    hidden_states = np.asarray(hidden_states, dtype=np.float32)
    cos = np.asarray(cos, dtype=np.float32)
    sin = np.asarray(sin, dtype=np.float32)
    Wqkv = np.asarray(Wqkv, dtype=np.float32)
    Wout = np.asarray(Wout, dtype=np.float32)
    gq = np.asarray(gq, dtype=np.float32)
    gk = np.asarray(gk, dtype=np.float32)

    if "nc" not in _CACHE:
        _CACHE["nc"] = _build_graph()
    nc = _CACHE["nc"]

    hsT = np.ascontiguousarray(hidden_states.reshape(TOK, C).T).astype(ml_dtypes.bfloat16)
    cosq = cos * gq[None, :]
    sinq = _fold_sin(sin, gq)
    cosk = cos * gk[None, :]
    sink = _fold_sin(sin, gk)
    trigc = np.concatenate([cosq, cosq, cosk, cosk], axis=1).astype(ml_dtypes.bfloat16)
    trigs = np.concatenate([sinq, sinq, sink, sink], axis=1).astype(ml_dtypes.bfloat16)
    wout_bf = Wout.astype(ml_dtypes.bfloat16)

    in_maps = []
    for c in range(NCORES):
        wq = Wqkv[:, c * 128:(c + 1) * 128]
        wk = Wqkv[:, C + c * 128:C + (c + 1) * 128]
        wv = Wqkv[:, 2 * C + c * 128:2 * C + (c + 1) * 128]
        wqkv_loc = np.ascontiguousarray(
            np.concatenate([wq, wk, wv], axis=1)).astype(ml_dtypes.bfloat16)
        in_maps.append({
            "hsT": hsT, "wqkv": wqkv_loc, "trigc": trigc, "trigs": trigs,
            "wout": wout_bf,
        })

    trace = bool(os.environ.get("BASS_TRACE"))
    res = run_bass_kernel_spmd(nc, in_maps, core_ids=list(range(NCORES)), trace=trace)
    _LAST_RESULT = res

    full = np.empty((B, N, C), dtype=np.float32)
    for c in range(NCORES):
        o = res.results[c]["out"]
        for b in range(B):
            full[b, c * 512:(c + 1) * 512, :] = o[b * 512:(b + 1) * 512]
    return full

